# revision 24
# baseline (speedup 1.0000x reference)
"""Chamfer-distance (bidirectional exact 1-NN) Trainium2 Bass kernel.

Problem: xyz1, xyz2 of shape [8, 4096, 3] fp32. For every point in cloud 1
find min/argmin squared distance to cloud 2 (dist1/idx1) and vice versa
(dist2/idx2), per batch.

Sharding: data-parallel over batch -- core b handles batch b (B=8=n_cores).
No cross-core communication.

Device algorithm per core (one batch), per direction:

1. Selection metric s[n,m] = 2*q_n.r_m - |r_m|^2 (= -d[n,m] + |q_n|^2; the
   q^2 term is independent of m so argmax_m s = argmin_m d). It is computed
   on the TensorEngine as ONE K=27 bf16 matmul per [128, 512] block:
   fp32 runs at 4 cycles/row on the PE but bf16 runs at 1, so each fp32
   operand is split exactly into three bf16 limbs (x = h + m + l, Dekker
   style, lossless for fp32) and the six significant limb products
   (hh', hm', mh', hl', lh', mm') are stacked along the contraction dim
   (3 coords x 6 = 18 rows). |r|^2 enters as 9 more rows: per-coordinate
   squares, computed and limb-split on device, paired against -1 rows.
   Dropped limb products are O(2^-24) relative -- the metric is fp32-class
   accurate and the PE runs 4x faster than an fp32 matmul.

2. For each 128-query tile: 8 matmuls fill PSUM [128, 4096], ScalarE copies
   to SBUF, then ONE custom VectorE op (running-max scan + record select +
   max-accumulate) returns argmax in a single pass (argmin of d).

3. The winning reference points are gathered with an indirect DMA and
   dist = sum((q - r*)^2) is recomputed in full fp32 by a fused custom op
   (matches the reference's maximum(d, 0) clamp since it is >= 0, and
   min_m max(d,0) == max(min_m d, 0)).

Host prep is lossless re-encoding only: batch slicing, transposes, and the
exact 3-limb bf16 split / doubling of input coordinates (h+m+l == x
bitwise). All arithmetic -- squares, their limb splits, distances,
argmins -- runs on device.

Argmax tie-breaking: the custom op returns the LAST index achieving the
running maximum; jnp.argmin returns the first. They differ only when two
reference points have bit-identical fp32 metric values -- vanishingly rare
(fp32 near-ties already flip ~0.2% of indices between ANY two
arithmetically different implementations, including reference vs float64).
"""

import numpy as np

B = 8
N = 4096
P = 128
NT = N // P  # 32 query tiles
CH = 512     # matmul free-dim chunk (one PSUM bank)

# limb-pair pattern: row j of each coordinate block pairs lhs limb L[j]
# with (doubled) rhs limb R[j]; products cover hh, hm, mh, hl, lh, mm.
_LHS_LIMB = [0, 0, 1, 0, 2, 1]  # 0=h 1=m 2=l
_RHS_LIMB = [0, 1, 0, 2, 0, 1]

_CACHE = {}


def _register_custom_ops():
    """Register the two custom DVE ops (idempotent). Returns (argmax_op, sqdiff_op)."""
    import concourse.dve_ops as dve_ops
    from concourse.dve_spec import (
        AluOp,
        Idx,
        MaxNeg,
        Spec,
        Src0,
        Src1,
        eq,
        lower,
        scan,
        select,
    )
    from concourse.dve_uop import DveOpSpec

    def _register(name, spec):
        if name in dve_ops._SUB_OPCODE_FOR_NAME:
            return next(o for o in dve_ops.OPS if o.name == name)
        row = dve_ops._CUSTOM_DVE_ROW_BASE + len(dve_ops.OPS)
        assert row < 0x20, "custom DVE opcode rows exhausted"
        dve_ops._SUB_OPCODE_FOR_NAME[name] = row
        op = dve_ops.DveOp(name, spec, subdim=False, uops_sha={})
        for ver in ("v3", "v4"):
            compiled = DveOpSpec(
                name=name,
                opcode=row,
                uops=lower(spec, ver=ver),
                rd1_en=dve_ops.has_src1(spec),
            )
            op.uops_sha[ver] = compiled.sha(ver)
        dve_ops.OPS.append(op)
        dve_ops.CUSTOM_DVE_SPECS[name] = spec
        return op

    def argmax_ref(in0, in1, c0, c1, c2):
        r = np.maximum.accumulate(in0, axis=-1)
        idx = np.arange(in0.shape[-1], dtype=np.float32)
        body = np.where(in0 == r, idx, -np.finfo(np.float32).max)
        return body, body.max(axis=-1, keepdims=True)

    argmax_op = _register(
        "ARGMAX_LASTREC_ANT",
        Spec(
            body=select(eq(Src0, scan(AluOp.MAX, Src0)), Idx, MaxNeg),
            accum=AluOp.MAX,
            reference=argmax_ref,
        ),
    )

    def sqdiff_ref(in0, in1, c0, c1, c2):
        body = (in0 - in1) * (in0 - in1)
        return body, body.sum(axis=-1, keepdims=True)

    sqdiff_op = _register(
        "SQDIFF_SUM_ANT",
        Spec(
            body=(Src0 - Src1) * (Src0 - Src1),
            accum=AluOp.ADD,
            reference=sqdiff_ref,
        ),
    )
    return argmax_op, sqdiff_op


def _build_program():
    from contextlib import ExitStack

    import concourse.bacc as bacc
    import concourse.bass as bass
    import concourse.mybir as mybir
    import concourse.tile as tile

    dt = mybir.dt
    Act = mybir.ActivationFunctionType

    argmax_op, sqdiff_op = _register_custom_ops()

    nc = bacc.Bacc("TRN2", target_bir_lowering=False, debug=False)

    # limb-expanded coordinate rows (host: lossless 3-limb bf16 split)
    qlh = nc.dram_tensor("qlh", [27, N], dt.bfloat16, kind="ExternalInput")
    qrh = nc.dram_tensor("qrh", [18, N], dt.bfloat16, kind="ExternalInput")
    rlh = nc.dram_tensor("rlh", [27, N], dt.bfloat16, kind="ExternalInput")
    rrh = nc.dram_tensor("rrh", [18, N], dt.bfloat16, kind="ExternalInput")
    q3 = nc.dram_tensor("q3", [3, N], dt.float32, kind="ExternalInput")
    r3 = nc.dram_tensor("r3", [3, N], dt.float32, kind="ExternalInput")
    qn = nc.dram_tensor("qn", [N, 3], dt.float32, kind="ExternalInput")
    rn = nc.dram_tensor("rn", [N, 3], dt.float32, kind="ExternalInput")
    d1 = nc.dram_tensor("d1", [P, NT], dt.float32, kind="ExternalOutput")
    d2 = nc.dram_tensor("d2", [P, NT], dt.float32, kind="ExternalOutput")
    i1 = nc.dram_tensor("i1", [P, NT], dt.int32, kind="ExternalOutput")
    i2 = nc.dram_tensor("i2", [P, NT], dt.int32, kind="ExternalOutput")

    with tile.TileContext(nc) as tc, ExitStack() as ctx:
        fixed = ctx.enter_context(tc.tile_pool(name="fixed", bufs=1))
        psum = ctx.enter_context(tc.tile_pool(name="psum", bufs=2, space="PSUM"))
        rows = ctx.enter_context(tc.tile_pool(name="rows", bufs=4))
        small = ctx.enter_context(tc.tile_pool(name="small", bufs=8))

        lhsq = fixed.tile([27, N], dt.bfloat16)  # q limbs + -1 rows
        rhsq = fixed.tile([27, N], dt.bfloat16)  # 2q limbs + |q|^2 limb rows
        lhsr = fixed.tile([27, N], dt.bfloat16)
        rhsr = fixed.tile([27, N], dt.bfloat16)
        scratch = fixed.tile([P, N], dt.float32)  # custom-op mandatory out
        sdiff = fixed.tile([P, 3], dt.float32)    # sqdiff mandatory out
        stg_d1 = fixed.tile([P, NT], dt.float32)
        stg_i1 = fixed.tile([P, NT], dt.uint32)
        stg_d2 = fixed.tile([P, NT], dt.float32)
        stg_i2 = fixed.tile([P, NT], dt.uint32)

        nc.sync.dma_start(lhsq[:], qlh.ap())
        nc.sync.dma_start(rhsq[0:18, :], qrh.ap())
        nc.sync.dma_start(lhsr[:], rlh.ap())
        nc.sync.dma_start(rhsr[0:18, :], rrh.ap())

        # per-coordinate squares of each cloud, limb-split on device, into
        # rows 18..26 of that cloud's rhs tile (paired against the -1 rows).
        # r first: direction A (lhsq x rhsr) is scheduled first and only
        # needs rhsr's square rows; q's chain overlaps direction A compute.
        # The dcmp pool is scoped to setup so its SBUF frees for row buffers.
        dcmp_cm = tc.tile_pool(name="dcmp", bufs=1)
        dcmp = dcmp_cm.__enter__()
        for c3d, rhs_t in ((r3, rhsr), (q3, rhsq)):
            c3 = dcmp.tile([3, N], dt.float32, tag="c3")
            nc.sync.dma_start(c3[:], c3d.ap())
            sqf = dcmp.tile([3, N], dt.float32, tag="sqf")
            nc.scalar.activation(sqf[:], c3[:], Act.Square)
            hh = dcmp.tile([3, N], dt.bfloat16, tag="hh")
            nc.scalar.copy(hh[:], sqf[:])
            t1 = dcmp.tile([3, N], dt.float32, tag="t1")
            nc.vector.tensor_tensor(
                t1[:], sqf[:], hh[:], mybir.AluOpType.subtract
            )
            mm = dcmp.tile([3, N], dt.bfloat16, tag="mm")
            nc.scalar.copy(mm[:], t1[:])
            t2 = dcmp.tile([3, N], dt.float32, tag="t2")
            nc.vector.tensor_tensor(
                t2[:], t1[:], mm[:], mybir.AluOpType.subtract
            )
            ll = dcmp.tile([3, N], dt.bfloat16, tag="ll")
            nc.scalar.copy(ll[:], t2[:])
            for k in range(3):
                for p, part in enumerate((hh, mm, ll)):
                    nc.sync.dma_start(
                        rhs_t[18 + 3 * k + p:19 + 3 * k + p, :],
                        part[k:k + 1, :],
                    )

        dcmp_cm.__exit__(None, None, None)

        for lhs_t, rhs_t, pts, q_pts, stg_d, stg_i in (
            (lhsq, rhsr, rn, qn, stg_d1, stg_i1),
            (lhsr, rhsq, qn, rn, stg_d2, stg_i2),
        ):
            for t in range(NT):
                row = rows.tile([P, N], dt.float32, tag="row")
                for h in range(2):
                    ps = psum.tile([P, 4 * CH], dt.float32, tag="mm")
                    for c in range(4):
                        k = h * 4 + c
                        nc.tensor.matmul(
                            ps[:, c * CH:(c + 1) * CH],
                            lhs_t[:, t * P:(t + 1) * P],
                            rhs_t[:, k * CH:(k + 1) * CH],
                            start=True,
                            stop=True,
                        )
                    nc.scalar.copy(row[:, h * 4 * CH:(h + 1) * 4 * CH], ps[:])
                # one-pass argmax = argmin(d), last-record index as f32
                af = small.tile([P, 1], dt.float32, tag="af")
                nc.vector._custom_dve(
                    argmax_op, out=scratch[:], in0=row[:], accum_out=af[:]
                )
                # stage index as uint32 (exact int conversion)
                nc.vector.tensor_copy(stg_i[:, t:t + 1], af[:])
                # gather winning reference points; recompute dist = sum((q-r*)^2)
                rg = small.tile([P, 3], dt.float32, tag="rg")
                nc.gpsimd.indirect_dma_start(
                    out=rg[:],
                    out_offset=None,
                    in_=pts.ap(),
                    in_offset=bass.IndirectOffsetOnAxis(
                        ap=stg_i[:, t:t + 1], axis=0
                    ),
                )
                qt = small.tile([P, 3], dt.float32, tag="qt")
                nc.sync.dma_start(qt[:], q_pts.ap()[t * P:(t + 1) * P, :])
                nc.vector._custom_dve(
                    sqdiff_op,
                    out=sdiff[:],
                    in0=qt[:],
                    in1=rg[:],
                    accum_out=stg_d[:, t:t + 1],
                )

        nc.sync.dma_start(d1.ap(), stg_d1[:])
        nc.sync.dma_start(d2.ap(), stg_d2[:])
        nc.sync.dma_start(i1.ap(), stg_i1[:].bitcast(dt.int32))
        nc.sync.dma_start(i2.ap(), stg_i2[:].bitcast(dt.int32))

    # Bacc compile legalizes multi-wait instructions (walrus accepts only a
    # single sync wait per instruction) via nop chains, plus DCE/nop-fusion.
    nc.compile()
    return nc


def _get_program():
    if "nc" not in _CACHE:
        _CACHE["nc"] = _build_program()
    return _CACHE["nc"]


def _limb_split(x):
    """Exact 3-limb bf16 split: x == h + m + l bitwise for fp32 input."""
    import ml_dtypes

    h = x.astype(ml_dtypes.bfloat16)
    res = x - h.astype(np.float32)
    m = res.astype(ml_dtypes.bfloat16)
    l = (res - m.astype(np.float32)).astype(ml_dtypes.bfloat16)
    return h, m, l


def _limb_rows(c3, doubled):
    """Build the bf16 limb-pattern rows for a [3, N] fp32 coord array.

    lhs pattern (doubled=False): [27, N] -- 18 limb rows plus 9 rows of -1
    (constant companions for the on-device |r|^2 limb rows).
    rhs pattern (doubled=True): [18, N] limb rows of 2*c3.
    """
    import ml_dtypes

    src = (c3 * 2.0) if doubled else c3
    limbs = _limb_split(src)  # tuple of three [3, N] bf16
    pattern = _RHS_LIMB if doubled else _LHS_LIMB
    nrows = 18 if doubled else 27
    out = np.full((nrows, c3.shape[1]), -1.0, dtype=ml_dtypes.bfloat16)
    for k in range(3):
        for j in range(6):
            out[6 * k + j] = limbs[pattern[j]][k]
    return out


def make_in_maps(xyz1, xyz2):
    xyz1 = np.asarray(xyz1, dtype=np.float32)
    xyz2 = np.asarray(xyz2, dtype=np.float32)
    in_maps = []
    for b in range(B):
        q3 = np.ascontiguousarray(xyz1[b].T)
        r3 = np.ascontiguousarray(xyz2[b].T)
        in_maps.append(
            {
                "qlh": _limb_rows(q3, doubled=False),
                "qrh": _limb_rows(q3, doubled=True),
                "rlh": _limb_rows(r3, doubled=False),
                "rrh": _limb_rows(r3, doubled=True),
                "q3": q3,
                "r3": r3,
                "qn": np.ascontiguousarray(xyz1[b]),
                "rn": np.ascontiguousarray(xyz2[b]),
            }
        )
    return in_maps


def unpack_outputs(results):
    d1 = np.stack([results[b]["d1"].T.reshape(-1) for b in range(B)])
    d2 = np.stack([results[b]["d2"].T.reshape(-1) for b in range(B)])
    i1 = np.stack([results[b]["i1"].T.reshape(-1) for b in range(B)])
    i2 = np.stack([results[b]["i2"].T.reshape(-1) for b in range(B)])
    return (
        d1.astype(np.float32),
        d2.astype(np.float32),
        i1.astype(np.int32),
        i2.astype(np.int32),
    )


def kernel(xyz1, xyz2):
    from concourse.bass_utils import run_bass_kernel_spmd

    nc = _get_program()
    in_maps = make_in_maps(xyz1, xyz2)
    res = run_bass_kernel_spmd(nc, in_maps, core_ids=list(range(B)))
    _CACHE["last_results"] = res
    return unpack_outputs(res.results)


# revision 31
# speedup vs baseline: 1.0666x; 1.0666x over previous
"""Chamfer-distance (bidirectional exact 1-NN) Trainium2 Bass kernel.

Problem: xyz1, xyz2 of shape [8, 4096, 3] fp32. For every point in cloud 1
find min/argmin squared distance to cloud 2 (dist1/idx1) and vice versa
(dist2/idx2), per batch.

Sharding: data-parallel over batch -- core b handles batch b (B=8=n_cores).
No cross-core communication.

Device algorithm per core (one batch), per direction:

1. Selection metric s[n,m] = 2*q_n.r_m - |r_m|^2 (= -d[n,m] + |q_n|^2; the
   q^2 term is independent of m so argmax_m s = argmin_m d). It is computed
   on the TensorEngine as ONE K=27 bf16 matmul per [128, 512] block:
   fp32 runs at 4 cycles/row on the PE but bf16 runs at 1, so each fp32
   operand is split exactly into three bf16 limbs (x = h + m + l, Dekker
   style, lossless for fp32) and the six significant limb products
   (hh', hm', mh', hl', lh', mm') are stacked along the contraction dim
   (3 coords x 6 = 18 rows). |r|^2 enters as 9 more rows: per-coordinate
   squares, computed and limb-split on device, paired against -1 rows.
   Dropped limb products are O(2^-24) relative -- the metric is fp32-class
   accurate and the PE runs 4x faster than an fp32 matmul.

2. For each 128-query tile: 8 matmuls fill PSUM [128, 4096], ScalarE copies
   to SBUF, then ONE custom VectorE op (running-max scan + record select +
   max-accumulate) returns argmax in a single pass (argmin of d).

3. The winning reference points are gathered with an indirect DMA and
   dist = sum((q - r*)^2) is recomputed in full fp32 by a fused custom op
   (matches the reference's maximum(d, 0) clamp since it is >= 0, and
   min_m max(d,0) == max(min_m d, 0)).

Host prep is lossless re-encoding only: batch slicing, transposes, and the
exact 3-limb bf16 split / doubling of input coordinates (h+m+l == x
bitwise). All arithmetic -- squares, their limb splits, distances,
argmins -- runs on device.

Argmax tie-breaking: the custom op returns the LAST index achieving the
running maximum; jnp.argmin returns the first. They differ only when two
reference points have bit-identical fp32 metric values -- vanishingly rare
(fp32 near-ties already flip ~0.2% of indices between ANY two
arithmetically different implementations, including reference vs float64).
"""

import numpy as np

B = 8
N = 4096
P = 128
NT = N // P  # 32 query tiles
CH = 512     # matmul free-dim chunk (one PSUM bank)

# limb-pair pattern: row j of each coordinate block pairs lhs limb L[j]
# with (doubled) rhs limb R[j]; products cover hh, hm, mh, hl, lh, mm.
_LHS_LIMB = [0, 0, 1, 0, 2, 1]  # 0=h 1=m 2=l
_RHS_LIMB = [0, 1, 0, 2, 0, 1]

_CACHE = {}


def _register_custom_ops():
    """Register the two custom DVE ops (idempotent). Returns (argmax_op, sqdiff_op)."""
    import concourse.dve_ops as dve_ops
    from concourse.dve_spec import (
        AluOp,
        Idx,
        MaxNeg,
        Spec,
        Src0,
        Src1,
        eq,
        lower,
        scan,
        select,
    )
    from concourse.dve_uop import DveOpSpec

    def _register(name, spec):
        if name in dve_ops._SUB_OPCODE_FOR_NAME:
            return next(o for o in dve_ops.OPS if o.name == name)
        row = dve_ops._CUSTOM_DVE_ROW_BASE + len(dve_ops.OPS)
        assert row < 0x20, "custom DVE opcode rows exhausted"
        dve_ops._SUB_OPCODE_FOR_NAME[name] = row
        op = dve_ops.DveOp(name, spec, subdim=False, uops_sha={})
        for ver in ("v3", "v4"):
            compiled = DveOpSpec(
                name=name,
                opcode=row,
                uops=lower(spec, ver=ver),
                rd1_en=dve_ops.has_src1(spec),
            )
            op.uops_sha[ver] = compiled.sha(ver)
        dve_ops.OPS.append(op)
        dve_ops.CUSTOM_DVE_SPECS[name] = spec
        return op

    def argmax_ref(in0, in1, c0, c1, c2):
        r = np.maximum.accumulate(in0, axis=-1)
        idx = np.arange(in0.shape[-1], dtype=np.float32)
        body = np.where(in0 == r, idx, -np.finfo(np.float32).max)
        return body, body.max(axis=-1, keepdims=True)

    argmax_op = _register(
        "ARGMAX_LASTREC_ANT",
        Spec(
            body=select(eq(Src0, scan(AluOp.MAX, Src0)), Idx, MaxNeg),
            accum=AluOp.MAX,
            reference=argmax_ref,
        ),
    )

    def sqdiff_ref(in0, in1, c0, c1, c2):
        body = (in0 - in1) * (in0 - in1)
        return body, body.sum(axis=-1, keepdims=True)

    sqdiff_op = _register(
        "SQDIFF_SUM_ANT",
        Spec(
            body=(Src0 - Src1) * (Src0 - Src1),
            accum=AluOp.ADD,
            reference=sqdiff_ref,
        ),
    )
    return argmax_op, sqdiff_op


def _build_program():
    from contextlib import ExitStack

    import concourse.bacc as bacc
    import concourse.bass as bass
    import concourse.mybir as mybir
    import concourse.tile as tile

    dt = mybir.dt
    Act = mybir.ActivationFunctionType

    argmax_op, sqdiff_op = _register_custom_ops()

    nc = bacc.Bacc("TRN2", target_bir_lowering=False, debug=False)

    # limb-expanded coordinate rows (host: lossless 3-limb bf16 split)
    qlh = nc.dram_tensor("qlh", [27, N], dt.bfloat16, kind="ExternalInput")
    qrh = nc.dram_tensor("qrh", [18, N], dt.bfloat16, kind="ExternalInput")
    rlh = nc.dram_tensor("rlh", [27, N], dt.bfloat16, kind="ExternalInput")
    rrh = nc.dram_tensor("rrh", [18, N], dt.bfloat16, kind="ExternalInput")
    # coords reshaped [48, 256]: partition 16k+a = coord k, sixteenth a --
    # setup ops use 48 partitions instead of 3 (16x fewer cycles, and the
    # square/limb-split dependency chain shortens to ~4us).
    q3 = nc.dram_tensor("q3", [48, N // 16], dt.float32, kind="ExternalInput")
    r3 = nc.dram_tensor("r3", [48, N // 16], dt.float32, kind="ExternalInput")
    qn = nc.dram_tensor("qn", [N, 3], dt.float32, kind="ExternalInput")
    rn = nc.dram_tensor("rn", [N, 3], dt.float32, kind="ExternalInput")
    d1 = nc.dram_tensor("d1", [P, NT], dt.float32, kind="ExternalOutput")
    d2 = nc.dram_tensor("d2", [P, NT], dt.float32, kind="ExternalOutput")
    i1 = nc.dram_tensor("i1", [P, NT], dt.int32, kind="ExternalOutput")
    i2 = nc.dram_tensor("i2", [P, NT], dt.int32, kind="ExternalOutput")

    with tile.TileContext(nc) as tc, ExitStack() as ctx:
        fixed = ctx.enter_context(tc.tile_pool(name="fixed", bufs=1))
        psum = ctx.enter_context(tc.tile_pool(name="psum", bufs=2, space="PSUM"))
        rows = ctx.enter_context(tc.tile_pool(name="rows", bufs=4))
        small = ctx.enter_context(tc.tile_pool(name="small", bufs=8))

        lhsq = fixed.tile([27, N], dt.bfloat16)  # q limbs + -1 rows
        rhsq = fixed.tile([27, N], dt.bfloat16)  # 2q limbs + |q|^2 limb rows
        lhsr = fixed.tile([27, N], dt.bfloat16)
        rhsr = fixed.tile([27, N], dt.bfloat16)
        scratch = fixed.tile([P, N], dt.float32)  # custom-op mandatory out
        sdiff = fixed.tile([P, 3], dt.float32)    # sqdiff mandatory out
        stg_d1 = fixed.tile([P, NT], dt.float32)
        stg_i1 = fixed.tile([P, NT], dt.uint32)
        stg_d2 = fixed.tile([P, NT], dt.float32)
        stg_i2 = fixed.tile([P, NT], dt.uint32)

        nc.sync.dma_start(lhsq[:], qlh.ap())
        nc.sync.dma_start(rhsq[0:18, :], qrh.ap())
        nc.sync.dma_start(lhsr[:], rlh.ap())
        nc.sync.dma_start(rhsr[0:18, :], rrh.ap())

        # per-coordinate squares of each cloud, limb-split on device, into
        # rows 18..26 of that cloud's rhs tile (paired against the -1 rows).
        # r first: direction A (lhsq x rhsr) is scheduled first and only
        # needs rhsr's square rows; q's chain overlaps direction A compute.
        # The dcmp pool is scoped to setup so its SBUF frees for row buffers.
        dcmp_cm = tc.tile_pool(name="dcmp", bufs=1)
        dcmp = dcmp_cm.__enter__()
        NQ = N // 16
        for c3d, rhs_t in ((r3, rhsr), (q3, rhsq)):
            c3 = dcmp.tile([48, NQ], dt.float32, tag="c3")
            nc.sync.dma_start(c3[:], c3d.ap())
            sqf = dcmp.tile([48, NQ], dt.float32, tag="sqf")
            nc.scalar.activation(sqf[:], c3[:], Act.Square)
            hh = dcmp.tile([48, NQ], dt.bfloat16, tag="hh")
            nc.scalar.copy(hh[:], sqf[:])
            t1 = dcmp.tile([48, NQ], dt.float32, tag="t1")
            nc.gpsimd.tensor_tensor(
                t1[:], sqf[:], hh[:], mybir.AluOpType.subtract
            )
            mm = dcmp.tile([48, NQ], dt.bfloat16, tag="mm")
            nc.scalar.copy(mm[:], t1[:])
            t2 = dcmp.tile([48, NQ], dt.float32, tag="t2")
            nc.gpsimd.tensor_tensor(
                t2[:], t1[:], mm[:], mybir.AluOpType.subtract
            )
            ll = dcmp.tile([48, NQ], dt.bfloat16, tag="ll")
            nc.scalar.copy(ll[:], t2[:])
            # row 18+3k+p of rhs = limb p of coord k: the 16 chunks of
            # coord k sit on contiguous partitions 16k..16k+15 -> one DMA.
            for k in range(3):
                for p, part in enumerate((hh, mm, ll)):
                    nc.sync.dma_start(
                        rhs_t[18 + 3 * k + p:19 + 3 * k + p, :].rearrange(
                            "r (a c) -> r a c", a=16
                        ),
                        part[16 * k:16 * k + 16, :],
                    )

        dcmp_cm.__exit__(None, None, None)

        rg_all1 = fixed.tile([P, NT, 3], dt.float32)
        qt_all1 = fixed.tile([P, NT, 3], dt.float32)
        rg_all2 = fixed.tile([P, NT, 3], dt.float32)
        qt_all2 = fixed.tile([P, NT, 3], dt.float32)
        df = fixed.tile([P, NT, 3], dt.float32)

        for lhs_t, rhs_t, pts, q_pts, stg_d, stg_i, rg_all, qt_all in (
            (lhsq, rhsr, rn, qn, stg_d1, stg_i1, rg_all1, qt_all1),
            (lhsr, rhsq, qn, rn, stg_d2, stg_i2, rg_all2, qt_all2),
        ):
            for t in range(NT):
                row = rows.tile([P, N], dt.float32, tag="row")
                for h in range(2):
                    ps = psum.tile([P, 4 * CH], dt.float32, tag="mm")
                    for c in range(4):
                        k = h * 4 + c
                        nc.tensor.matmul(
                            ps[:, c * CH:(c + 1) * CH],
                            lhs_t[:, t * P:(t + 1) * P],
                            rhs_t[:, k * CH:(k + 1) * CH],
                            start=True,
                            stop=True,
                        )
                    nc.scalar.copy(row[:, h * 4 * CH:(h + 1) * 4 * CH], ps[:])
                # one-pass argmax = argmin(d), last-record index as f32
                af = small.tile([P, 1], dt.float32, tag="af")
                nc.vector._custom_dve(
                    argmax_op, out=scratch[:], in0=row[:], accum_out=af[:]
                )
                # stage index as uint32 (exact int conversion) on gpsimd
                nc.gpsimd.tensor_copy(stg_i[:, t:t + 1], af[:])
                # gather winning reference points into the per-direction pool
                nc.gpsimd.indirect_dma_start(
                    out=rg_all[:, t],
                    out_offset=None,
                    in_=pts.ap(),
                    in_offset=bass.IndirectOffsetOnAxis(
                        ap=stg_i[:, t:t + 1], axis=0
                    ),
                )
                nc.sync.dma_start(
                    qt_all[:, t], q_pts.ap()[t * P:(t + 1) * P, :]
                )
            # batched dist = sum((q - r*)^2) for the whole direction
            nc.vector.tensor_tensor(
                df[:], qt_all[:], rg_all[:], mybir.AluOpType.subtract
            )
            nc.vector.tensor_tensor(df[:], df[:], df[:], mybir.AluOpType.mult)
            nc.vector.tensor_reduce(
                stg_d[:], df[:], axis=mybir.AxisListType.X,
                op=mybir.AluOpType.add,
            )

        nc.sync.dma_start(d1.ap(), stg_d1[:])
        nc.sync.dma_start(d2.ap(), stg_d2[:])
        nc.sync.dma_start(i1.ap(), stg_i1[:].bitcast(dt.int32))
        nc.sync.dma_start(i2.ap(), stg_i2[:].bitcast(dt.int32))

    # Bacc compile legalizes multi-wait instructions (walrus accepts only a
    # single sync wait per instruction) via nop chains, plus DCE/nop-fusion.
    nc.compile()
    return nc


def _get_program():
    if "nc" not in _CACHE:
        _CACHE["nc"] = _build_program()
    return _CACHE["nc"]


def _limb_split(x):
    """Exact 3-limb bf16 split: x == h + m + l bitwise for fp32 input."""
    import ml_dtypes

    h = x.astype(ml_dtypes.bfloat16)
    res = x - h.astype(np.float32)
    m = res.astype(ml_dtypes.bfloat16)
    l = (res - m.astype(np.float32)).astype(ml_dtypes.bfloat16)
    return h, m, l


def _limb_rows(c3, doubled):
    """Build the bf16 limb-pattern rows for a [3, N] fp32 coord array.

    lhs pattern (doubled=False): [27, N] -- 18 limb rows plus 9 rows of -1
    (constant companions for the on-device |r|^2 limb rows).
    rhs pattern (doubled=True): [18, N] limb rows of 2*c3.
    """
    import ml_dtypes

    src = (c3 * 2.0) if doubled else c3
    limbs = _limb_split(src)  # tuple of three [3, N] bf16
    pattern = _RHS_LIMB if doubled else _LHS_LIMB
    nrows = 18 if doubled else 27
    out = np.full((nrows, c3.shape[1]), -1.0, dtype=ml_dtypes.bfloat16)
    for k in range(3):
        for j in range(6):
            out[6 * k + j] = limbs[pattern[j]][k]
    return out


def make_in_maps(xyz1, xyz2):
    xyz1 = np.asarray(xyz1, dtype=np.float32)
    xyz2 = np.asarray(xyz2, dtype=np.float32)
    in_maps = []
    for b in range(B):
        q3 = np.ascontiguousarray(xyz1[b].T)
        r3 = np.ascontiguousarray(xyz2[b].T)
        # [3, N] -> [48, N//16]: partition 16k+a = coord k, sixteenth a
        q3q = np.ascontiguousarray(q3.reshape(48, N // 16))
        r3q = np.ascontiguousarray(r3.reshape(48, N // 16))
        in_maps.append(
            {
                "qlh": _limb_rows(q3, doubled=False),
                "qrh": _limb_rows(q3, doubled=True),
                "rlh": _limb_rows(r3, doubled=False),
                "rrh": _limb_rows(r3, doubled=True),
                "q3": q3q,
                "r3": r3q,
                "qn": np.ascontiguousarray(xyz1[b]),
                "rn": np.ascontiguousarray(xyz2[b]),
            }
        )
    return in_maps


def unpack_outputs(results):
    d1 = np.stack([results[b]["d1"].T.reshape(-1) for b in range(B)])
    d2 = np.stack([results[b]["d2"].T.reshape(-1) for b in range(B)])
    i1 = np.stack([results[b]["i1"].T.reshape(-1) for b in range(B)])
    i2 = np.stack([results[b]["i2"].T.reshape(-1) for b in range(B)])
    return (
        d1.astype(np.float32),
        d2.astype(np.float32),
        i1.astype(np.int32),
        i2.astype(np.int32),
    )


def kernel(xyz1, xyz2):
    from concourse.bass_utils import run_bass_kernel_spmd

    nc = _get_program()
    in_maps = make_in_maps(xyz1, xyz2)
    res = run_bass_kernel_spmd(nc, in_maps, core_ids=list(range(B)))
    _CACHE["last_results"] = res
    return unpack_outputs(res.results)
